# revision 5
# baseline (speedup 1.0000x reference)
"""Trainium2 Bass kernel: GatedRecurrentCell, v2 (decoupled-PE pipeline).

Math (per batch b, channel n, time t):
    pa = x @ Wa^T ; pi = x @ Wi^T
    sig_i = sigmoid(pi + bi);  w = (pi + bi) * sig_i          (silu)
    t_a = tanh((pa + ba)/2)   -> sigmoid(pa+ba) = (t_a+1)/2
    a   = exp(-ln3/2 * t_a + ln(sigmoid(gate)) - ln3/2)       (= alpha*3^-sig)
    c   = sqrt(1 - a^2) * w
    h_t = a_t h_{t-1} + c_t ;  out = h

Engine mapping (per core = one batch):
    PE   : pi/pa GEMMs, full 2048-wide rows, ping-pong in PSUM (4+4 banks)
    DVE  : PSUM evacuation with affine fold  u2=(pi*0.5+bi/2), v=(pa*0.5+ba/2)
           (both bf16, so PE never waits on ACT's table phases), w=(t_i+1)*u2
    ACT  : t_i=Tanh(u2), t_a=Tanh(v) [exp-set], a=Exp(-ln3/2*t_a+lnam) per-ic
           [exp-set], q=Sqrt(1-a2) [sqrt-set, batched every 2 rounds]
           -> only exp/sqrt table sets, ~2+ loads total instead of 27
    Pool : a2=a*a, c=q*w (in place), per-channel-row scans
    silu is rewritten via tanh so the silu table set is never loaded:
           silu(z) = z*sigmoid(z) = (z/2)*(1+tanh(z/2)) = u2*(t_i+1)

All intermediates and the output are bf16 (scan state stays fp32 inside the
DVE/Pool scan instruction); host upcasts.
"""

import functools
import os

import numpy as np

B, S, D, I = 8, 2048, 512, 2048
P = 128
NCORES = 8
LN3 = float(np.log(3.0))

# ics per round (ACT batch width = G*S)
G_ROUND = int(os.environ.get("GRC_G", "2"))
# batch sqrt over this many rounds (amortizes table switches)
Q_LAZY = int(os.environ.get("GRC_QLAZY", "2"))
# GEMM operand dtype: "f32r" (exact-ish) or "bf16" (halves input DMA)
X_DT = os.environ.get("GRC_XDT", "f32r")


def _build_nc(s, d, i, g_round=G_ROUND, q_lazy=Q_LAZY, x_dt=X_DT):
    import concourse.bacc as bacc
    import concourse.mybir as mybir
    import concourse.tile as tile
    from concourse.tile import add_dep_helper

    F32 = mybir.dt.float32
    BF16 = mybir.dt.bfloat16
    F32R = mybir.dt.float32r
    AF = mybir.ActivationFunctionType
    ALU = mybir.AluOpType

    MMDT = mybir.dt.bfloat16 if x_dt == "bf16" else mybir.dt.float32r
    MMIO = mybir.dt.bfloat16 if x_dt == "bf16" else mybir.dt.float32

    nd = d // P
    ni = i // P
    g = min(g_round, ni)
    # ramped round sizes: small first round (ACT starts sooner) and a
    # small last round (short scan/DMA tail)
    if ni >= 6 and g >= 2:
        sizes = [1]
        while sum(sizes) + g <= ni - 1:
            sizes.append(g)
        sizes += [1] * (ni - sum(sizes))
    else:
        sizes = [min(g, ni - k) for k in range(0, ni, g)]
    assert sum(sizes) == ni

    nc = bacc.Bacc("TRN2", target_bir_lowering=False, debug=False,
                   num_devices=NCORES)

    xT_d = nc.dram_tensor("xT", [d, s], MMIO, kind="ExternalInput").ap()
    waT_d = nc.dram_tensor("WaT", [ni, P, d], MMIO,
                           kind="ExternalInput").ap()
    wiT_d = nc.dram_tensor("WiT", [ni, P, d], MMIO,
                           kind="ExternalInput").ap()
    bih_d = nc.dram_tensor("bihT", [P, ni], F32, kind="ExternalInput").ap()
    bah_d = nc.dram_tensor("bahT", [P, ni], F32, kind="ExternalInput").ap()
    lnam_d = nc.dram_tensor("lnamT", [P, ni], F32, kind="ExternalInput").ap()
    out_d = nc.dram_tensor("out", [i, s], BF16, kind="ExternalOutput").ap()

    with tile.TileContext(nc) as tc:
        from contextlib import ExitStack

        with ExitStack() as ctx:
            const_pool = ctx.enter_context(tc.tile_pool(name="const", bufs=1))
            xt_pool = ctx.enter_context(tc.tile_pool(name="xt", bufs=1))
            wst_pool = ctx.enter_context(tc.tile_pool(name="wstream", bufs=1))
            ps_pool = ctx.enter_context(
                tc.tile_pool(name="mmpsum", bufs=1, space="PSUM"))
            row_pool = ctx.enter_context(tc.tile_pool(name="rows", bufs=1))
            h_pool = ctx.enter_context(tc.tile_pool(name="hout", bufs=1))

            # ---- input DMA preamble, ordered for earliest first GEMM:
            # ic0 weights -> first x column strip -> consts -> more
            # weights -> remaining strips --------------------------------
            w_sb_pre = {}

            def w_dma(ic):
                wi_sb = wst_pool.tile([P, d], MMDT, name=f"wi{ic}",
                                      tag="wi", bufs=3)
                nc.sync.dma_start(wi_sb[:], wiT_d[ic].bitcast(MMDT))
                wa_sb = wst_pool.tile([P, d], MMDT, name=f"wa{ic}",
                                      tag="wa", bufs=3)
                nc.sync.dma_start(wa_sb[:], waT_d[ic].bitcast(MMDT))
                w_sb_pre[ic] = (wi_sb, wa_sb)

            xT_sb = []
            for k in range(nd):
                xT_sb.append(
                    xt_pool.tile([P, s], MMDT, name=f"xT{k}", tag=f"xT{k}"))
            xcw = 512

            def strip_dma(hcol):
                for k in range(nd):
                    nc.sync.dma_start(
                        xT_sb[k][:, hcol * xcw:(hcol + 1) * xcw],
                        xT_d[k * P:(k + 1) * P,
                             hcol * xcw:(hcol + 1) * xcw].bitcast(MMDT))

            w_dma(0)
            strip_dma(0)

            bih_t = const_pool.tile([P, ni], F32, name="bih_t")
            nc.sync.dma_start(bih_t[:], bih_d[:])
            bah_t = const_pool.tile([P, ni], F32, name="bah_t")
            nc.sync.dma_start(bah_t[:], bah_d[:])
            lnam_t = const_pool.tile([P, ni], F32, name="lnam_t")
            nc.sync.dma_start(lnam_t[:], lnam_d[:])

            for hcol in range(1, s // xcw):
                strip_dma(hcol)
            for ic in (1, 2):
                if ic < ni:
                    w_dma(ic)

            def gemm(ps, w_sb, m0=0, m1=None):
                if m1 is None:
                    m1 = s // 512
                for m in range(m0, m1):
                    for k in range(nd):
                        nc.tensor.matmul(
                            ps[:, m * 512:(m + 1) * 512],
                            w_sb[:, k * P:(k + 1) * P],
                            xT_sb[k][:, m * 512:m * 512 + 512],
                            start=(k == 0), stop=(k == nd - 1))

            # ---- ACT program-order chain (pins table-set phases) --------
            act_chain = []

            def act(out_ap, in_ap, func, **kw):
                inst = nc.scalar.activation(out_ap, in_ap, func, **kw)
                if act_chain:
                    add_dep_helper(inst.ins, act_chain[-1].ins, False,
                                   "act table phase order")
                act_chain.append(inst)
                return inst

            # ---- main loop ----------------------------------------------
            rounds = []
            pos = 0
            for sz in sizes:
                rounds.append(list(range(pos, pos + sz)))
                pos += sz
            # flush (sqrt + tail) schedule: pairs of rounds; when the count
            # is odd, the straggler flushes ALONE mid-stream and the final
            # two rounds pair up, so the last flush's exp-set work stays
            # contiguous with the preceding round (one sqrt load at the end)
            nrounds = len(rounds)
            if nrounds % 2 == 0:
                flush_after = set(range(1, nrounds, 2))
            elif nrounds >= 3:
                flush_after = set(range(1, nrounds - 3, 2))
                flush_after |= {nrounds - 3, nrounds - 1}
            else:
                flush_after = {nrounds - 1}
            # deferred sqrt work: list of (round_idx, a2_buf, q_buf)
            pend_q = []
            # per-round state kept until the sqrt/c/scan tail runs
            state = {}

            def flush_q(final=False):
                """Emit batched sqrt for pending rounds, then c + scans."""
                for (ridx, s_b, q_b, gs) in pend_q:
                    act(q_b[:, :gs], s_b[:, :gs], AF.Sqrt,
                        scale=-1.0, bias=1.0)
                for n_, (ridx, s_b, q_b, gs) in enumerate(pend_q):
                    a_b, w_b, ics_r = state.pop(ridx)
                    last_of_all = final and n_ == len(pend_q) - 1
                    # the very last round's c goes to DVE (idle by then) so
                    # it runs concurrently with Pool's previous-round tail;
                    # scans are ALWAYS DVE (the scan opcode is illegal on
                    # Pool)
                    ceng = nc.vector if last_of_all else nc.gpsimd
                    # c = q*w (in place over w)
                    ceng.tensor_mul(w_b[:, :gs], q_b[:, :gs], w_b[:, :gs])
                    for j, ic in enumerate(ics_r):
                        h_t = h_pool.tile([P, s], BF16, name=f"h{ic}",
                                          tag="h", bufs=3)
                        if last_of_all and j == len(ics_r) - 1:
                            # final channel row: half-row scans chained via
                            # the AP initial so the first half's output DMA
                            # overlaps the second half's scan
                            hh = s // 2
                            lo = j * s
                            nc.vector.tensor_tensor_scan(
                                h_t[:, :hh], a_b[:, lo:lo + hh],
                                w_b[:, lo:lo + hh], 0.0,
                                op0=ALU.mult, op1=ALU.add)
                            nc.scalar.dma_start(
                                out_d[ic * P:(ic + 1) * P, :hh],
                                h_t[:, :hh])
                            nc.vector.tensor_tensor_scan(
                                h_t[:, hh:], a_b[:, lo + hh:lo + s],
                                w_b[:, lo + hh:lo + s],
                                h_t[:, hh - 1:hh],
                                op0=ALU.mult, op1=ALU.add)
                            nc.scalar.dma_start(
                                out_d[ic * P:(ic + 1) * P, hh:],
                                h_t[:, hh:])
                            continue
                        nc.vector.tensor_tensor_scan(
                            h_t[:], a_b[:, j * s:(j + 1) * s],
                            w_b[:, j * s:(j + 1) * s], 0.0,
                            op0=ALU.mult, op1=ALU.add)
                        nc.sync.dma_start(out_d[ic * P:(ic + 1) * P, :],
                                          h_t[:])
                pend_q.clear()

            warmed = False

            for ridx, ics in enumerate(rounds):
                gs = len(ics) * s
                gmax = g * s

                def rtile(tag):
                    return row_pool.tile([P, gmax], BF16,
                                         name=f"{tag}_{ridx}", tag=tag,
                                         bufs=2)

                u2_b = rtile("u2")
                v_b = rtile("v")
                ti_b = rtile("ti")
                ta_b = rtile("ta")
                a_b = rtile("a")
                w_b = rtile("w")
                s_b = rtile("s")
                q_b = rtile("q")

                # --- PE + DVE evacuation ---------------------------------
                # all pi GEMMs of the round first (t_i can start after
                # ics-count GEMMs instead of 2*ics-1), then all pa
                w_sbs = {}
                for ic in ics:
                    if ic in w_sb_pre:
                        w_sbs[ic] = w_sb_pre[ic]
                    else:
                        wi_sb = wst_pool.tile([P, d], MMDT, name=f"wi{ic}",
                                              tag="wi", bufs=3)
                        nc.sync.dma_start(wi_sb[:], wiT_d[ic].bitcast(MMDT))
                        wa_sb = wst_pool.tile([P, d], MMDT, name=f"wa{ic}",
                                              tag="wa", bufs=3)
                        nc.sync.dma_start(wa_sb[:], waT_d[ic].bitcast(MMDT))
                        w_sbs[ic] = (wi_sb, wa_sb)

                # round 0 runs at half-row granularity so the first ACT
                # work starts as soon as the first x column strips land
                halves = ([(0, s // 1024), (s // 1024, s // 512)]
                          if ridx == 0 else [(0, s // 512)])
                for j, ic in enumerate(ics):
                    # pi/pa ping-pong per ic: the pa GEMM covers the pi
                    # evacuation (and vice versa), so PE never stalls on
                    # the single-buffered PSUM tiles
                    pi_ps = ps_pool.tile([P, s], F32, name=f"pi{ic}",
                                         tag="pi", bufs=1)
                    if not warmed:
                        # dummy matmuls into the first pi tile: PE activity
                        # during the x/W input DMA so the HAM clock gate
                        # opens before the first real GEMM (whose
                        # start=True resets the accumulators)
                        warm_w = const_pool.tile([P, P], MMIO,
                                                 name="warmw")
                        nc.vector.memset(warm_w[:], 0.0)
                        warm_x = const_pool.tile([P, 512], MMIO,
                                                 name="warmx")
                        nc.vector.memset(warm_x[:], 0.0)
                        for _ in range(20):
                            nc.tensor.matmul(pi_ps[:, 0:512],
                                             warm_w[:].bitcast(MMDT),
                                             warm_x[:].bitcast(MMDT),
                                             start=True, stop=True)
                        warmed = True
                    # u2 = (pi * 0.5 + bi/2)  [DVE, PSUM->SBUF bf16]
                    for (m0, m1) in halves:
                        gemm(pi_ps, w_sbs[ic][0], m0, m1)
                        nc.vector.tensor_scalar(
                            u2_b[:, j * s + m0 * 512:j * s + m1 * 512],
                            pi_ps[:, m0 * 512:m1 * 512],
                            0.5, bih_t[:, ic:ic + 1], ALU.mult, ALU.add)
                    pa_ps = ps_pool.tile([P, s], F32, name=f"pa{ic}",
                                         tag="pa", bufs=1)
                    # v = (pa * 0.5 + ba/2)  [DVE]
                    for (m0, m1) in halves:
                        gemm(pa_ps, w_sbs[ic][1], m0, m1)
                        nc.vector.tensor_scalar(
                            v_b[:, j * s + m0 * 512:j * s + m1 * 512],
                            pa_ps[:, m0 * 512:m1 * 512],
                            0.5, bah_t[:, ic:ic + 1], ALU.mult, ALU.add)

                # --- ACT phase [exp set]: t_i, t_a, a --------------------
                if ridx == 0:
                    # half-row instructions chase the progressive GEMM
                    hw_ = s // 2
                    for lo in range(0, gs, hw_):
                        act(ti_b[:, lo:lo + hw_], u2_b[:, lo:lo + hw_],
                            AF.Tanh)
                    for lo in range(0, gs, hw_):
                        act(ta_b[:, lo:lo + hw_], v_b[:, lo:lo + hw_],
                            AF.Tanh)
                else:
                    act(ti_b[:, :gs], u2_b[:, :gs], AF.Tanh)
                    act(ta_b[:, :gs], v_b[:, :gs], AF.Tanh)
                for j, ic in enumerate(ics):
                    act(a_b[:, j * s:(j + 1) * s],
                        ta_b[:, j * s:(j + 1) * s], AF.Exp,
                        scale=-LN3 / 2.0, bias=lnam_t[:, ic:ic + 1])

                # --- Pool: w = (t_i + 1) * u2  (exact silu) --------------
                # (Pool, not DVE: DVE's strict FIFO must stay pure-evac so
                #  the next round's PSUM frees are never queued behind an
                #  ACT-dependent op. Two plain TensorTensor ops — Pool has
                #  no TensorScalarPtr opcode.)
                nc.gpsimd.tensor_mul(w_b[:, :gs], ti_b[:, :gs],
                                     u2_b[:, :gs])
                nc.gpsimd.tensor_add(w_b[:, :gs], w_b[:, :gs],
                                     u2_b[:, :gs])
                # --- Pool: a2 = a*a --------------------------------------
                nc.gpsimd.tensor_mul(s_b[:, :gs], a_b[:, :gs], a_b[:, :gs])

                state[ridx] = (a_b, w_b, ics)
                pend_q.append((ridx, s_b, q_b, gs))
                # NOTE: never let more than 2 rounds pend — round tiles have
                # bufs=2, and a 3rd pending round's allocs would wait on
                # releases that sit after this round's ops in the pinned ACT
                # chain (deadlock).
                if ridx in flush_after:
                    flush_q(final=(ridx == len(rounds) - 1))

    nc.compile()
    return nc


@functools.lru_cache(maxsize=2)
def _get_nc(s=S, d=D, i=I):
    return _build_nc(s, d, i)


LAST_RESULTS = None


def _prep_core_inputs(xb, WaT, WiT, bihT, bahT, lnamT):
    return {"xT": np.ascontiguousarray(xb.T), "WaT": WaT, "WiT": WiT,
            "bihT": bihT, "bahT": bahT, "lnamT": lnamT}


def _prep_shared(Wa, ba, Wi, bi, gate, d, i):
    ni = i // P
    nd = d // P
    WaT = np.ascontiguousarray(
        Wa.reshape(ni, P, nd, P).transpose(0, 3, 2, 1).reshape(ni, P, d))
    WiT = np.ascontiguousarray(
        Wi.reshape(ni, P, nd, P).transpose(0, 3, 2, 1).reshape(ni, P, d))
    bihT = np.ascontiguousarray((bi * 0.5).reshape(ni, P).T)
    bahT = np.ascontiguousarray((ba * 0.5).reshape(ni, P).T)
    g64 = gate.astype(np.float64)
    lnam = np.log(1.0 / (1.0 + np.exp(-g64))) - LN3 / 2.0
    lnamT = np.ascontiguousarray(
        lnam.astype(np.float32).reshape(ni, P).T)
    return WaT, WiT, bihT, bahT, lnamT


def kernel(x, Wa, ba, Wi, bi, gate):
    global LAST_RESULTS
    from concourse.bass_utils import run_bass_kernel_spmd

    x = np.asarray(x, dtype=np.float32)
    b, s, d = x.shape
    i = Wa.shape[0]
    nc = _get_nc(s, d, i)

    WaT, WiT, bihT, bahT, lnamT = _prep_shared(
        np.asarray(Wa, np.float32), np.asarray(ba, np.float32),
        np.asarray(Wi, np.float32), np.asarray(bi, np.float32),
        np.asarray(gate, np.float32), d, i)

    in_maps = [_prep_core_inputs(x[bb], WaT, WiT, bihT, bahT, lnamT)
               for bb in range(b)]
    res = run_bass_kernel_spmd(nc, in_maps, list(range(b)))
    LAST_RESULTS = res
    out = np.stack(
        [np.asarray(res.results[bb]["out"]).astype(np.float32).T
         for bb in range(b)], axis=0)
    return np.ascontiguousarray(out, dtype=np.float32)
